# revision 72
# baseline (speedup 1.0000x reference)
"""GAT 2-layer distributed Bass kernel for TRN2 (8 cores) — v3.

Changes vs v2 baseline:
  - ONE AllGather of the full layer-2 table (rank-major row order) instead
    of 7 chunked AllGathers: small chunks ran at ~40GB/s with a 15us fixed
    cost each, serialized on the collective cores (~1ms total).
  - rank-major global order requires per-core ROTATED local order for T1
    (own shard first), so the dense phase + layer-1 gathers use per-core
    local row ids (input data), layer-2 gathers use global row ids. The
    per-layer edge->slot assignments (A/B views) therefore differ.
  - dst-attention gathers read own-shard local rows: prefetched and
    compacted to 8 cols, overlapping the dense phase (layer 1, via the
    small ownT side table) / the AllGather window (layer 2).
  - software-pipelined epilogue (deferred one tile) so the in-order DVE
    never head-of-line blocks on the current tile's PSUM.
  - bf16 epilogue chain with the PSUM read on the Act engine, and a
    stride-1-pair formulation of the one-hot is_equal: every hot-loop DVE
    op runs in 2x mode.
  - no reserved dummy slots: pad edges point at row 0; the one-hot
    scatter (dstloc=-1 never matches a lane) masks their contribution.

Table layout per node-slot row (768B = 384 bf16, RB; gather elem and row
stride must both be 256B multiples):
  [asrc 8 | adst 8 | h 256 | pad 112] (all bf16)
T1 = layer-1 table, per-core LOCAL order (written by replicated dense)
cc_all = this core's own-shard layer-2 rows (local order, rows 0..SH)
T2 = AllGather(cc_all) -> global rank-major order

Global row id: k*SH + t*128 + p. Core k's local row of node (j,t,p):
((j-k) mod 8)*SH + t*128 + p.
"""
import dataclasses
import numpy as np


@dataclasses.dataclass
class Cfg:
    ncores: int = 8
    ntile: int = 49          # dst tiles per core
    nchunk: int = 9          # 128-edge chunks per tile
    na: int = 512            # A-view edge slots per tile
    nb: int = 640            # B-view edge slots per tile
    n: int = 50000           # real nodes
    e: int = 400000
    fin: int = 128
    h: int = 8
    c: int = 32              # layer-1 head dim (h*c = 256)
    out: int = 32            # layer-2 head dim
    bt: int = 4              # tiles per gather batch
    use_collective: bool = True
    xbatch: int = 56         # dense node-tiles per x-stream DMA
    wg: int = 28             # T1 row-groups per write DMA
    ccb: int = 7             # tiles per cc staging batch

    @property
    def shslots(self):
        return self.ntile * 128

    @property
    def nslot(self):
        return self.ncores * self.shslots

    @property
    def va(self):
        return min(32256, self.nslot)

    @property
    def vb(self):
        return self.nslot - self.va

    @property
    def d1(self):
        return self.h * self.c      # 256

    @property
    def rec(self):
        # 192 f32 per row (768B): gather elem AND row stride must both be
        # multiples of 256B, and [asrc 8|adst 8|h 256] = 544B doesn't fit 512B
        return 16 + self.d1 // 2 + 48


def host_prep(cfg: Cfg, x, edge_index, W1, as1, ad1, b1, W2, as2, ad2, b2):
    N, E = cfg.n, cfg.e
    NC, NT, SH = cfg.ncores, cfg.ntile, cfg.shslots
    src = np.asarray(edge_index[0], dtype=np.int64)
    dst = np.asarray(edge_index[1], dtype=np.int64)
    deg = np.bincount(dst, minlength=N)

    # ---- assign nodes to (core, tile, slot), balancing edge counts ----
    order = np.argsort(-deg, kind="stable")
    core_load = np.zeros(NC, dtype=np.int64)
    core_cnt = np.zeros(NC, dtype=np.int64)
    cap_core = N // NC
    node_core = np.empty(N, dtype=np.int64)
    for nd in order:
        k = np.argmin(np.where(core_cnt < cap_core, core_load, np.iinfo(np.int64).max))
        node_core[nd] = k
        core_load[k] += deg[nd]
        core_cnt[k] += 1

    # node -> (core, tile, lane)
    node_k = np.empty(N, dtype=np.int64)
    node_t = np.empty(N, dtype=np.int64)
    node_p = np.empty(N, dtype=np.int64)
    slot2node = np.full(cfg.nslot, -1, dtype=np.int64)
    for k in range(NC):
        nodes_k = order[node_core[order] == k]
        tcap = np.full(NT, 128, dtype=np.int64)
        tload = np.zeros(NT, dtype=np.int64)
        tcnt = np.zeros(NT, dtype=np.int64)
        tmember = [[] for _ in range(NT)]
        for nd in nodes_k:
            t = np.argmin(np.where(tcnt < tcap, tload, np.iinfo(np.int64).max))
            tmember[t].append(nd)
            tload[t] += deg[nd]
            tcnt[t] += 1
        for t in range(NT):
            for i, nd in enumerate(tmember[t]):
                node_k[nd], node_t[nd], node_p[nd] = k, t, i
                slot2node[k * SH + t * 128 + i] = nd

    grow = node_k * SH + node_t * 128 + node_p     # global row per node

    # ---- per (core, tile) edge lists with per-layer A/B split ----
    ecore = node_k[dst]
    etile = node_t[dst]
    dlane = node_p[dst]

    NA, NB, NCH = cfg.na, cfg.nb, cfg.nchunk
    assert NA + NB == NCH * 128 and NA % 128 == 0 and NB % 128 == 0

    # layer 1 uses local rows (rotate rank blocks by -k), layer 2 global
    idxA = np.zeros((2, NC, NT, NA), dtype=np.int64)
    idxB = np.zeros((2, NC, NT, NB), dtype=np.int64)
    idxD = np.zeros((2, NC, NT, NA + NB), dtype=np.int64)  # local dst row
    dloc = np.full((2, NC, NT, NA + NB), -1.0, dtype=np.float32)

    for k in range(NC):
        srow_local = ((node_k[src] - k) % NC) * SH + node_t[src] * 128 + node_p[src]
        srow_global = grow[src]
        for lay, srow in ((0, srow_local), (1, srow_global)):
            for t in range(NT):
                sel = np.nonzero((ecore == k) & (etile == t))[0]
                ss = srow[sel]
                inA = ss < cfg.va
                inB = ss >= cfg.vb
                flex = inA & inB
                a_list = list(np.nonzero(inA & ~inB)[0])
                b_list = list(np.nonzero(inB & ~inA)[0])
                for i in np.nonzero(flex)[0]:
                    if NA - len(a_list) >= NB - len(b_list):
                        a_list.append(i)
                    else:
                        b_list.append(i)
                if len(a_list) > NA or len(b_list) > NB:
                    raise RuntimeError(
                        f"tile overflow l{lay} core{k} tile{t}: "
                        f"{len(a_list)}/{NA} {len(b_list)}/{NB}")
                # pad slots: negative tail indices -> the gather skips them
                # (their SBUF slots stay stale; dstloc=-1 masks their
                # contribution and a one-time memset guards against NaNs)
                idxA[lay, k, t, len(a_list):] = -1
                idxB[lay, k, t, len(b_list):] = -1
                for p, i in enumerate(a_list):
                    e_id = sel[i]
                    idxA[lay, k, t, p] = ss[i]
                    idxD[lay, k, t, p] = t * 128 + dlane[e_id]
                    dloc[lay, k, t, p] = dlane[e_id]
                for p, i in enumerate(b_list):
                    e_id = sel[i]
                    idxB[lay, k, t, p] = ss[i] - cfg.vb
                    idxD[lay, k, t, NA + p] = t * 128 + dlane[e_id]
                    dloc[lay, k, t, NA + p] = dlane[e_id]

    def pack16(v, width):
        # v [.., NT, width] int -> [.., NT, 128, width//16] int16 wrapped+replicated
        assert v.shape[-1] == width and width % 16 == 0
        lead = v.shape[:-2]
        r = v.reshape(*lead, -1, width // 16, 16)
        r = np.moveaxis(r, -1, -2)      # [.., NT, 16, width//16]
        reps = (1,) * (len(lead) + 1) + (8, 1)
        r = np.tile(r, reps).astype(np.int16)
        return np.ascontiguousarray(r)

    pidxA = pack16(idxA, NA)
    pidxB = pack16(idxB, NB)
    pidxD = pack16(idxD, NA + NB)

    import ml_dtypes
    # dstloc [2, NC, NT, 128, NCH, 2]: position p = j*128 + lane; pads -> -1.
    # Each value is duplicated into a stride-1 pair so the one-hot is_equal
    # can present a stride-1 last dim on every operand (DVE 2x mode).
    dloc_t = np.moveaxis(dloc.reshape(2, NC, NT, NCH, 128), -1, -2)
    dloc_t = np.repeat(dloc_t[..., None], 2, axis=-1)
    dloc_t = np.ascontiguousarray(dloc_t.astype(ml_dtypes.bfloat16))

    # validmask [NC, NT, 128, 1] indexed by (core, tile, lane)
    vm = np.zeros((NC, NT, 128, 1), dtype=np.float32)
    for k in range(NC):
        for t in range(NT):
            vm[k, t, :, 0] = slot2node[k * SH + t * 128 + np.arange(128)] >= 0
    vm = np.ascontiguousarray(vm)

    # x permuted to global row order, then per-core rotation of rank blocks
    xp = np.zeros((cfg.nslot, cfg.fin), dtype=np.float32)
    real = slot2node >= 0
    xp[real] = np.asarray(x, dtype=np.float32)[slot2node[real]]
    xp = xp.astype(ml_dtypes.bfloat16)
    xTs = []
    for k in range(NC):
        xk = np.roll(xp.reshape(NC, SH, cfg.fin), -k, axis=0).reshape(
            cfg.nslot, cfg.fin)
        xTs.append(np.ascontiguousarray(xk.T))

    def chmaj(M, hdim, axis):
        # permute an (h, c)-ordered head block to (c, h) order along axis
        M = np.moveaxis(M, axis, 0)
        M = M.reshape(cfg.h, hdim, *M.shape[1:])
        M = np.swapaxes(M, 0, 1).reshape(cfg.h * hdim, *M.shape[2:])
        return np.moveaxis(M, 0, axis)

    def fuse(W, asv, adv, hdim):
        Wa = np.einsum("fhc,hc->fh", W.reshape(W.shape[0], cfg.h, hdim), asv)
        Wd = np.einsum("fhc,hc->fh", W.reshape(W.shape[0], cfg.h, hdim), adv)
        # attn columns first so the dense phase can copy [attn|h] in one op;
        # h-part columns in (c h) order so the DVE msg multiply gets
        # stride-1 last dims on every operand (2x/4x fast path)
        return np.concatenate([Wa, Wd, chmaj(W, hdim, 1)], axis=1)

    Wall1 = fuse(np.asarray(W1, np.float32), np.asarray(as1, np.float32),
                 np.asarray(ad1, np.float32), cfg.c).astype(ml_dtypes.bfloat16)
    Wall2f = fuse(chmaj(np.asarray(W2, np.float32), cfg.c, 0),
                  np.asarray(as2, np.float32),
                  np.asarray(ad2, np.float32), cfg.out)
    Wall2 = np.ascontiguousarray(
        Wall2f.reshape(2, 128, Wall2f.shape[1])).astype(ml_dtypes.bfloat16)

    b1t = np.ascontiguousarray(np.tile(
        chmaj(np.asarray(b1, np.float32), cfg.c, 0),
        (128, 1))).astype(ml_dtypes.bfloat16)
    b2t = np.ascontiguousarray(np.tile(np.asarray(b2, np.float32), (128, 1)))
    iota = np.tile(np.arange(128, dtype=np.float32), (128, 1)).astype(ml_dtypes.bfloat16)
    ident = np.eye(128, dtype=np.float32).astype(ml_dtypes.bfloat16)

    # out2 row (k, t*128+p) -> node id (-1 for padding lanes)
    out_node = np.full((NC, SH), -1, dtype=np.int64)
    for k in range(NC):
        out_node[k] = slot2node[k * SH:(k + 1) * SH]

    in_maps = []
    for k in range(NC):
        in_maps.append({
            "xT": xTs[k], "Wall1": Wall1, "Wall2": Wall2,
            "b1t": b1t, "b2t": b2t, "iota": iota, "ident": ident,
            "idxA1": pidxA[0, k], "idxB1": pidxB[0, k], "idxD1": pidxD[0, k],
            "idxA2": pidxA[1, k], "idxB2": pidxB[1, k], "idxD2": pidxD[1, k],
            "dstloc1": dloc_t[0, k], "dstloc2": dloc_t[1, k],
            "vmask": vm[k],
        })
    return in_maps, out_node


def build(cfg: Cfg):
    import concourse.bacc as bacc
    import concourse.mybir as mybir
    import concourse.tile as tile
    from concourse import library_config
    from contextlib import ExitStack

    f32 = mybir.dt.float32
    bf16 = mybir.dt.bfloat16
    i16 = mybir.dt.int16
    AOP = mybir.AluOpType
    ACTF = mybir.ActivationFunctionType
    X = mybir.AxisListType.X

    NC, NT, NCH, NA, NB = cfg.ncores, cfg.ntile, cfg.nchunk, cfg.na, cfg.nb
    NAC, NBC = NA // 128, NB // 128
    SH, NSLOT, VA, VB = cfg.shslots, cfg.nslot, cfg.va, cfg.vb
    D1, REC, BT = cfg.d1, cfg.rec, cfg.bt
    HE = 16 + D1          # written row prefix (bf16 cols) = 272
    NW = D1 + 16          # fused weight cols = 272
    RB = 2 * REC          # row length in bf16 units (288 = 576B)
    EPS = 1e-16
    CCB = cfg.ccb

    nc = bacc.Bacc('TRN2', target_bir_lowering=False, debug=False, num_devices=NC)

    xT_d = nc.dram_tensor('xT', [128, NSLOT], bf16, kind='ExternalInput')
    Wall1_d = nc.dram_tensor('Wall1', [128, NW], bf16, kind='ExternalInput')
    Wall2_d = nc.dram_tensor('Wall2', [2, 128, NW], bf16, kind='ExternalInput')
    b1t_d = nc.dram_tensor('b1t', [128, D1], bf16, kind='ExternalInput')
    b2t_d = nc.dram_tensor('b2t', [128, cfg.out], f32, kind='ExternalInput')
    iota_d = nc.dram_tensor('iota', [128, 128], bf16, kind='ExternalInput')
    ident_d = nc.dram_tensor('ident', [128, 128], bf16, kind='ExternalInput')
    idx_d = {}
    for lay in (1, 2):
        idx_d[f'idxA{lay}'] = nc.dram_tensor(
            f'idxA{lay}', [NT, 128, NA // 16], i16, kind='ExternalInput')
        idx_d[f'idxB{lay}'] = nc.dram_tensor(
            f'idxB{lay}', [NT, 128, NB // 16], i16, kind='ExternalInput')
        idx_d[f'idxD{lay}'] = nc.dram_tensor(
            f'idxD{lay}', [NT, 128, (NA + NB) // 16], i16, kind='ExternalInput')
        idx_d[f'dstloc{lay}'] = nc.dram_tensor(
            f'dstloc{lay}', [NT, 128, NCH, 2], bf16, kind='ExternalInput')
    vmask_d = nc.dram_tensor('vmask', [NT, 128, 1], f32, kind='ExternalInput')
    out2_d = nc.dram_tensor('out2', [SH, cfg.out], f32, kind='ExternalOutput')
    T1 = nc.dram_tensor('T1', [NSLOT, RB], bf16, kind='Internal')
    # own-shard [asrc|adst|h-head] cols, written early in the dense phase so
    # the layer-1 dst-attention prefetch overlaps the rest of dense
    ownT = nc.dram_tensor('ownT', [SH, 128], bf16, kind='Internal')
    cc_all = nc.dram_tensor('cc_all', [SH, RB], bf16, kind='Internal')
    T2 = nc.dram_tensor('T2', [NSLOT, RB], bf16, kind='Internal',
                        addr_space='Shared' if cfg.use_collective else 'Local')

    with tile.TileContext(nc) as tc, ExitStack() as ctx, \
            nc.allow_low_precision(reason="bf16 epilogue; rel-err gate 2e-2"):
        const = ctx.enter_context(tc.tile_pool(name='const', bufs=1))
        nc.gpsimd.load_library(library_config.mlp)

        w1 = const.tile([128, NW], bf16)
        nc.sync.dma_start(w1[:], Wall1_d[:])
        w2 = const.tile([128, 2, NW], bf16)
        nc.sync.dma_start(w2[:], Wall2_d[:].rearrange("k p w -> p k w"))
        b1 = const.tile([128, D1], bf16)
        nc.sync.dma_start(b1[:], b1t_d[:])
        b2 = const.tile([128, cfg.out], f32)
        nc.sync.dma_start(b2[:], b2t_d[:])
        iot = const.tile([128, 128], bf16)
        nc.sync.dma_start(iot[:], iota_d[:])
        idn = const.tile([128, 128], bf16)
        nc.sync.dma_start(idn[:], ident_d[:])
        vmt = const.tile([128, NT], f32)
        nc.sync.dma_start(vmt[:], vmask_d[:].rearrange("t p o -> p (t o)"))

        # persistent pool for the dst-attention prefetch gathers: allocated
        # BEFORE the dense-phase pools so its SBUF does not alias theirs
        # (aliasing would add an anti-dependency that kills the overlap)
        pfp = ctx.enter_context(tc.tile_pool(name='pf', bufs=2))
        PBT = 4

        # ---------- edge phases ----------
        idxp = ctx.enter_context(tc.tile_pool(name='idx', bufs=1))

        def load_idx(lay):
            iaL = idxp.tile([128, NT, NA // 16], i16, name=f'iaL{lay}')
            nc.sync.dma_start(iaL[:], idx_d[f'idxA{lay}'][:].rearrange("t p w -> p t w"))
            ibL = idxp.tile([128, NT, NB // 16], i16, name=f'ibL{lay}')
            nc.sync.dma_start(ibL[:], idx_d[f'idxB{lay}'][:].rearrange("t p w -> p t w"))
            idL = idxp.tile([128, NT, (NA + NB) // 16], i16, name=f'idL{lay}')
            nc.sync.dma_start(idL[:], idx_d[f'idxD{lay}'][:].rearrange("t p w -> p t w"))
            dlL = idxp.tile([128, NT, NCH, 2], bf16, name=f'dlL{lay}')
            nc.sync.dma_start(dlL[:], idx_d[f'dstloc{lay}'][:].rearrange(
                "t p w two -> p t w two"))
            return iaL, ibL, idL, dlL

        idx1 = load_idx(1)

        # per-edge dst attention, compacted to 8 cols: zda[lay][:, t, j, h]
        zda = {1: idxp.tile([128, NT, NCH, 8], bf16, name='zda1'),
               2: idxp.tile([128, NT, NCH, 8], bf16, name='zda2')}


        # ---------- phase D1: replicated dense, writes T1 (local order) ----
        with tc.tile_pool(name='dx', bufs=2) as dx, \
             tc.tile_pool(name='dps', bufs=4, space='PSUM') as dps, \
             tc.tile_pool(name='dstg', bufs=3) as dstg:
            ng = NSLOT // 128
            for g0 in range(0, ng, cfg.xbatch):
                gb = min(cfg.xbatch, ng - g0)
                xt = dx.tile([128, gb * 128], bf16, tag='xt')
                nc.sync.dma_start(xt[:], xT_d[:, g0 * 128:(g0 + gb) * 128])
                for w0 in range(0, gb, cfg.wg):
                    wg = min(cfg.wg, gb - w0)
                    stg = dstg.tile([128, wg, HE], bf16, tag=f'stg{wg}')
                    for t in range(wg):
                        ps = dps.tile([128, NW], f32, tag='dps')
                        nc.tensor.matmul(ps[:], xt[:, (w0 + t) * 128:(w0 + t + 1) * 128],
                                         w1[:], start=True, stop=True)
                        if t % 2 == 0:
                            nc.scalar.copy(stg[:, t, :], ps[:])
                        else:
                            nc.vector.tensor_copy(stg[:, t, :], ps[:])
                    g = g0 + w0
                    nc.sync.dma_start(
                        T1[g * 128:(g + wg) * 128, 0:HE].rearrange(
                            "(t p) r -> p t r", p=128),
                        stg[:])
                    if g < NT:
                        gc = min(wg, NT - g)
                        nc.sync.dma_start(
                            ownT[g * 128:(g + gc) * 128, :].rearrange(
                                "(t p) r -> p t r", p=128),
                            stg[:, 0:gc, 0:128])

        def prefetch_gd_batch(layer, adst_src, estep, idL, b0):
            """Gather adst[dst] for one batch of edge slots (256B elems from
            the local own-shard rows; cols 8:16 are the payload) and compact
            into zda. Runs on Pool/DMA only."""
            bt = min(PBT, NT - b0)
            gD = pfp.tile([128, PBT * NCH, 128], bf16, tag='gD')
            nc.gpsimd.dma_gather(
                gD[:, 0:bt * NCH, :], adst_src,
                idL[:, b0:b0 + bt, :].rearrange("p t w -> p (t w)"),
                bt * (NA + NB), bt * (NA + NB), 128, elem_step=estep,
                single_packet=False)
            nc.scalar.copy(
                zda[layer][:, b0:b0 + bt, :, :],
                gD[:, 0:bt * NCH, 8:16].rearrange(
                    "p (t j) h -> p t j h", t=bt))

        def edge_phase(layer, T, idxt, epilogue, prefetch_cb=None):
            iaL, ibL, idL, dlL = idxt
            zd = zda[layer]
            pname = f'e{layer}'
            pend = [None]
            GBUFS = 4
            with tc.tile_pool(name=pname + 'g', bufs=GBUFS) as gp, \
                 tc.tile_pool(name=pname + 'w', bufs=4) as wp, \
                 tc.tile_pool(name=pname + 'o', bufs=4) as op, \
                 tc.tile_pool(name=pname + 'ps', bufs=4, space='PSUM') as pp:
                rowA_src = T[0:VA, :]
                rowB_src = T[VB:NSLOT, :]

                # one-time memset of every gather buffer: pad tail slots are
                # skipped by the DMA (negative idx) and must not hold NaNs
                # (0 x NaN would poison the PSUM scatter)
                for _ in range(GBUFS):
                    gz = gp.tile([128, NCH, RB], bf16, tag='gA')
                    nc.gpsimd.memset(gz[:], 0.0)

                if True:
                    for tg in range(NT):
                        if prefetch_cb is not None and tg % BT == 0:
                            prefetch_cb(tg)
                        # both view gathers land in ONE tile (disjoint row
                        # ranges) so zb/msg each run as a single DVE op
                        g = gp.tile([128, NCH, RB], bf16, tag='gA')
                        nc.gpsimd.dma_gather(
                            g[:, 0:NAC, :], rowA_src,
                            iaL[:, tg, :],
                            NA, NA, RB, single_packet=False)
                        nc.gpsimd.dma_gather(
                            g[:, NAC:NCH, :], rowB_src,
                            ibL[:, tg, :],
                            NB, NB, RB, single_packet=False)
                        # one-hots for all chunks of this tile: oh[e, j, slot]
                        ohs = wp.tile([128, NCH, 128], bf16, tag='ohs')
                        # stride-1 pair view on every operand -> DVE 2x mode
                        nc.vector.tensor_tensor(
                            ohs[:].rearrange("p j (s two) -> p j s two", two=2),
                            iot[:].rearrange("p (s two) -> p () s two", two=2)
                            .to_broadcast([128, NCH, 64, 2]),
                            dlL[:, tg, :, :].rearrange("p j two -> p j () two")
                            .to_broadcast([128, NCH, 64, 2]),
                            op=AOP.is_equal)
                        # z = asrc[src] + adst[dst]; leaky; exp
                        zb = wp.tile([128, NCH * 8], bf16, tag='zb')
                        nc.vector.tensor_tensor(
                            zb[:].rearrange("p (b h) -> p b h", b=NCH),
                            g[:, :, 0:8],
                            zd[:, tg, :, :],
                            op=AOP.add)
                        zl = wp.tile([128, NCH * 8], bf16, tag='zl')
                        nc.vector.scalar_tensor_tensor(
                            zl[:], zb[:], 0.2, zb[:], op0=AOP.mult, op1=AOP.max)
                        p = wp.tile([128, NCH * 8], bf16, tag='p')
                        nc.scalar.activation(p[:], zl[:], ACTF.Exp)
                        # msg = h[src] * p  (h stored (c h)-major: all
                        # operands stride-1 in the last dim -> DVE fast path)
                        msg = wp.tile([128, NCH, 32, 8], bf16, tag='msgA')
                        nc.vector.tensor_tensor(
                            msg[:],
                            g[:, :, 16:16 + D1].rearrange(
                                "p b (c h) -> p b c h", h=8),
                            p[:].rearrange(
                                "p (b h) -> p b () h", b=NCH).to_broadcast(
                                [128, NCH, 32, 8]),
                            op=AOP.mult)
                        # scatter to dst slots
                        paw = pp.tile([128, D1], f32, tag='paw')
                        pdt = pp.tile([128, 8], f32, tag='aux', name='pdt')
                        for j in range(NCH):
                            rhs = msg[:, j].rearrange("p c h -> p (c h)")
                            nc.tensor.matmul(paw[:], ohs[:, j, :], rhs,
                                             start=(j == 0), stop=(j == NCH - 1))
                            nc.tensor.matmul(
                                pdt[:], ohs[:, j, :], p[:, j * 8:(j + 1) * 8],
                                start=(j == 0), stop=(j == NCH - 1))
                        # software-pipelined epilogue: defer by one tile so
                        # the in-order DVE never head-of-line blocks on this
                        # tile's PSUM while the next tile's front work is ready
                        if pend[0] is not None:
                            epilogue(*pend[0])
                        pend[0] = (tg, paw[:], pdt[:], op, pp)
                if pend[0] is not None:
                    epilogue(*pend[0])
                    pend[0] = None

        # ---------- epilogues ----------
        ccstage = {}

        def epi1(tg, pa, pd, op, pp):
            d1 = op.tile([128, 8], f32, tag='d1')
            # PSUM-near Act engine does the EPS add (Copy table is
            # always resident); keeps the DVE chain off PSUM reads
            nc.scalar.activation(d1[:], pd, ACTF.Copy, bias=EPS)
            r = op.tile([128, 8], bf16, tag='r')
            nc.vector.reciprocal(r[:], d1[:])
            # PSUM read on Act (bf16 out) so the DVE chain stays in 2x mode
            pab = op.tile([128, D1], bf16, tag='pab')
            nc.scalar.copy(pab[:], pa)
            o1 = op.tile([128, D1], bf16, tag='o1')
            rb = r[:].rearrange("p h -> p () h").to_broadcast([128, 32, 8])
            nc.vector.tensor_tensor(o1[:].rearrange("p (c h) -> p c h", h=8),
                                    pab[:].rearrange("p (c h) -> p c h", h=8),
                                    rb, op=AOP.mult)
            nc.vector.tensor_tensor(o1[:], o1[:], b1[:], op=AOP.add)
            ex = op.tile([128, D1], bf16, tag='ex')
            nc.scalar.activation(ex[:], o1[:], ACTF.Exp)
            nc.vector.tensor_scalar(ex[:], ex[:], 1.0, 1.0, op0=AOP.min,
                                    op1=AOP.subtract)
            et = op.tile([128, D1], bf16, tag='et')
            nc.vector.scalar_tensor_tensor(
                et[:], o1[:], 0.0, ex[:], op0=AOP.max, op1=AOP.add)
            # ---- fused D2: h2 row for this tile -> cc staging ----
            lh = op.tile([128, 2, 128], bf16, tag='lh')
            ptr = pp.tile([128, 2, 128], bf16, tag='aux', name='ptr')
            nc.tensor.transpose(ptr[:, 0], et[:, 0:128], idn[:])
            nc.tensor.transpose(ptr[:, 1], et[:, 128:256], idn[:])
            nc.scalar.copy(lh[:], ptr[:])
            pd2 = pp.tile([128, NW], f32, tag='aux', name='pd2')
            nc.tensor.matmul(pd2[:], lh[:, 0], w2[:, 0], start=True, stop=False)
            nc.tensor.matmul(pd2[:], lh[:, 1], w2[:, 1], start=False, stop=True)
            # per-tile cc write (544B payload cols only; cc_all pad cols stay
            # DRAM-stale, nothing consumes them): the last tile's rows reach
            # DRAM right after its epilogue, so the AllGather starts sooner
            row = op.tile([128, HE], bf16, tag='ccstg')
            nc.scalar.activation(row[:], pd2[:],
                                 ACTF.Copy, scale=vmt[:, tg:tg + 1])
            nc.sync.dma_start(
                cc_all[tg * 128:(tg + 1) * 128, 0:HE].rearrange(
                    "(t p) r -> p (t r)", p=128), row[:])

        outstage = {}

        def epi2(tg, pa, pd, op, pp):
            d1 = op.tile([128, 8], f32, tag='d1')
            # PSUM-near Act engine does the EPS add (Copy table is
            # always resident); keeps the DVE chain off PSUM reads
            nc.scalar.activation(d1[:], pd, ACTF.Copy, bias=EPS)
            r = op.tile([128, 8], bf16, tag='r')
            nc.vector.reciprocal(r[:], d1[:])
            pab = op.tile([128, D1], bf16, tag='pab')
            nc.scalar.copy(pab[:], pa)
            o1 = op.tile([128, D1], bf16, tag='o1')
            rb = r[:].rearrange("p h -> p () h").to_broadcast([128, cfg.out, 8])
            nc.vector.tensor_tensor(o1[:].rearrange("p (c h) -> p c h", h=8),
                                    pab[:].rearrange("p (c h) -> p c h", h=8),
                                    rb, op=AOP.mult)
            m = op.tile([128, cfg.out], f32, tag='m')
            nc.vector.reduce_sum(m[:].rearrange("p c -> p c ()"),
                                 o1[:].rearrange("p (c h) -> p c h", h=8), axis=X)
            # per-tile out write: the kernel end isn't gated on a 7-tile
            # staging group after the last epilogue
            ob = op.tile([128, cfg.out], f32, tag='ostg')
            nc.vector.scalar_tensor_tensor(ob[:], m[:], 1.0 / cfg.h,
                                           b2[:], op0=AOP.mult, op1=AOP.add)
            nc.sync.dma_start(
                out2_d[tg * 128:(tg + 1) * 128, :].rearrange(
                    "(t p) c -> p (t c)", p=128), ob[:])

        # layer-1 dst-attention prefetch: the first batches depend only on
        # the early ownT writes and overlap the rest of the dense phase;
        # later batches issue inside the L1 loop (two batches ahead) so
        # their DMA rides L1's slack instead of delaying the T1 writes
        # that gate L1's first row gathers
        prefetch_gd_batch(1, ownT[0:SH, :], 128, idx1[2], 0)
        prefetch_gd_batch(1, ownT[0:SH, :], 128, idx1[2], PBT)

        def pf1_cb(b0):
            nxt = b0 + 2 * PBT
            if nxt < NT:
                prefetch_gd_batch(1, ownT[0:SH, :], 128, idx1[2], nxt)

        edge_phase(1, T1, idx1, epi1, prefetch_cb=pf1_cb)

        if cfg.use_collective:
            nc.gpsimd.collective_compute(
                "AllGather", mybir.AluOpType.bypass,
                ins=[cc_all[:]],
                outs=[T2[:]],
                replica_groups=[list(range(NC))],
            )
        else:
            # timing-sim-only stand-in for the AllGather: copy local rows to
            # every rank block of T2 (values wrong cross-core, local DMA cost
            # similar to the receive side of the real collective)
            with tc.tile_pool(name='ccb', bufs=2) as ccbp:
                for cci in range(NT // CCB):
                    bb = ccbp.tile([128, CCB, RB], bf16, tag='bb')
                    nc.sync.dma_start(
                        bb[:], cc_all[cci * CCB * 128:(cci + 1) * CCB * 128,
                                      :].rearrange("(t p) r -> p t r", p=128))
                    for k in range(NC):
                        nc.sync.dma_start(
                            T2[k * SH + cci * CCB * 128:
                               k * SH + (cci + 1) * CCB * 128, :].rearrange(
                                "(t p) r -> p t r", p=128), bb[:])

        # layer-2 index loads + dst-attention prefetch: issued after the
        # AllGather so both run inside its window (the prefetch reads only
        # the local cc staging)
        idx2 = load_idx(2)
        for b0 in range(0, NT, PBT):
            prefetch_gd_batch(2, cc_all[0:SH, 0:128], RB, idx2[2], b0)

        edge_phase(2, T2, idx2, epi2)

    nc.compile()
    return nc


_CACHE = {}


def kernel(x, edge_index, W1, att_src1, att_dst1, b1, W2, att_src2,
           att_dst2, b2):
    cfg = Cfg()
    in_maps, out_node = host_prep(cfg, x, edge_index, W1, att_src1,
                                  att_dst1, b1, W2, att_src2, att_dst2, b2)
    if 'nc' not in _CACHE:
        _CACHE['nc'] = build(cfg)
    nc = _CACHE['nc']
    from concourse.bass_utils import run_bass_kernel_spmd
    res = run_bass_kernel_spmd(nc, in_maps, core_ids=list(range(cfg.ncores)))
    full = np.concatenate([res.results[k]['out2'] for k in range(cfg.ncores)],
                          axis=0)
    flat = out_node.reshape(-1)
    out = np.zeros((cfg.n, cfg.out), np.float32)
    real = flat >= 0
    out[flat[real]] = full[real]
    return out


# revision 75
# speedup vs baseline: 1.0370x; 1.0370x over previous
"""GAT 2-layer distributed Bass kernel for TRN2 (8 cores) — v3.

Changes vs v2 baseline:
  - ONE AllGather of the full layer-2 table (rank-major row order) instead
    of 7 chunked AllGathers: small chunks ran at ~40GB/s with a 15us fixed
    cost each, serialized on the collective cores (~1ms total).
  - rank-major global order requires per-core ROTATED local order for T1
    (own shard first), so the dense phase + layer-1 gathers use per-core
    local row ids (input data), layer-2 gathers use global row ids. The
    per-layer edge->slot assignments (A/B views) therefore differ.
  - dst-attention gathers read own-shard local rows: prefetched and
    compacted to 8 cols, overlapping the dense phase (layer 1, via the
    small ownT side table) / the AllGather window (layer 2).
  - software-pipelined epilogue (deferred one tile) so the in-order DVE
    never head-of-line blocks on the current tile's PSUM.
  - bf16 epilogue chain with the PSUM read on the Act engine, and a
    stride-1-pair formulation of the one-hot is_equal: every hot-loop DVE
    op runs in 2x mode.
  - no reserved dummy slots: pad edges point at row 0; the one-hot
    scatter (dstloc=-1 never matches a lane) masks their contribution.

Table layout per node-slot row (768B = 384 bf16, RB; gather elem and row
stride must both be 256B multiples):
  [asrc 8 | adst 8 | h 256 | pad 112] (all bf16)
T1 = layer-1 table, per-core LOCAL order (written by replicated dense)
cc_all = this core's own-shard layer-2 rows (local order, rows 0..SH)
T2 = AllGather(cc_all) -> global rank-major order

Global row id: k*SH + t*128 + p. Core k's local row of node (j,t,p):
((j-k) mod 8)*SH + t*128 + p.
"""
import dataclasses
import numpy as np


@dataclasses.dataclass
class Cfg:
    ncores: int = 8
    ntile: int = 49          # dst tiles per core
    nchunk: int = 9          # 128-edge chunks per tile
    na: int = 512            # A-view edge slots per tile
    nb: int = 640            # B-view edge slots per tile
    n: int = 50000           # real nodes
    e: int = 400000
    fin: int = 128
    h: int = 8
    c: int = 32              # layer-1 head dim (h*c = 256)
    out: int = 32            # layer-2 head dim
    bt: int = 4              # tiles per gather batch
    use_collective: bool = True
    xbatch: int = 56         # dense node-tiles per x-stream DMA
    wg: int = 28             # T1 row-groups per write DMA
    ccb: int = 7             # tiles per cc staging batch

    @property
    def shslots(self):
        return self.ntile * 128

    @property
    def nslot(self):
        return self.ncores * self.shslots

    @property
    def va(self):
        return min(32256, self.nslot)

    @property
    def vb(self):
        return self.nslot - self.va

    @property
    def d1(self):
        return self.h * self.c      # 256

    @property
    def rec(self):
        # 192 f32 per row (768B): gather elem AND row stride must both be
        # multiples of 256B, and [asrc 8|adst 8|h 256] = 544B doesn't fit 512B
        return 16 + self.d1 // 2 + 48


def host_prep(cfg: Cfg, x, edge_index, W1, as1, ad1, b1, W2, as2, ad2, b2):
    N, E = cfg.n, cfg.e
    NC, NT, SH = cfg.ncores, cfg.ntile, cfg.shslots
    src = np.asarray(edge_index[0], dtype=np.int64)
    dst = np.asarray(edge_index[1], dtype=np.int64)
    deg = np.bincount(dst, minlength=N)

    # ---- assign nodes to (core, tile, slot), balancing edge counts ----
    order = np.argsort(-deg, kind="stable")
    core_load = np.zeros(NC, dtype=np.int64)
    core_cnt = np.zeros(NC, dtype=np.int64)
    cap_core = N // NC
    node_core = np.empty(N, dtype=np.int64)
    for nd in order:
        k = np.argmin(np.where(core_cnt < cap_core, core_load, np.iinfo(np.int64).max))
        node_core[nd] = k
        core_load[k] += deg[nd]
        core_cnt[k] += 1

    # node -> (core, tile, lane)
    node_k = np.empty(N, dtype=np.int64)
    node_t = np.empty(N, dtype=np.int64)
    node_p = np.empty(N, dtype=np.int64)
    slot2node = np.full(cfg.nslot, -1, dtype=np.int64)
    for k in range(NC):
        nodes_k = order[node_core[order] == k]
        tcap = np.full(NT, 128, dtype=np.int64)
        tload = np.zeros(NT, dtype=np.int64)
        tcnt = np.zeros(NT, dtype=np.int64)
        tmember = [[] for _ in range(NT)]
        for nd in nodes_k:
            t = np.argmin(np.where(tcnt < tcap, tload, np.iinfo(np.int64).max))
            tmember[t].append(nd)
            tload[t] += deg[nd]
            tcnt[t] += 1
        for t in range(NT):
            for i, nd in enumerate(tmember[t]):
                node_k[nd], node_t[nd], node_p[nd] = k, t, i
                slot2node[k * SH + t * 128 + i] = nd

    grow = node_k * SH + node_t * 128 + node_p     # global row per node

    # ---- per (core, tile) edge lists with per-layer A/B split ----
    ecore = node_k[dst]
    etile = node_t[dst]
    dlane = node_p[dst]

    NA, NB, NCH = cfg.na, cfg.nb, cfg.nchunk
    assert NA + NB == NCH * 128 and NA % 128 == 0 and NB % 128 == 0

    # layer 1 uses local rows (rotate rank blocks by -k), layer 2 global
    idxA = np.zeros((2, NC, NT, NA), dtype=np.int64)
    idxB = np.zeros((2, NC, NT, NB), dtype=np.int64)
    idxD = np.zeros((2, NC, NT, NA + NB), dtype=np.int64)  # local dst row
    dloc = np.full((2, NC, NT, NA + NB), -1.0, dtype=np.float32)

    for k in range(NC):
        srow_local = ((node_k[src] - k) % NC) * SH + node_t[src] * 128 + node_p[src]
        srow_global = grow[src]
        for lay, srow in ((0, srow_local), (1, srow_global)):
            for t in range(NT):
                sel = np.nonzero((ecore == k) & (etile == t))[0]
                ss = srow[sel]
                inA = ss < cfg.va
                inB = ss >= cfg.vb
                flex = inA & inB
                a_list = list(np.nonzero(inA & ~inB)[0])
                b_list = list(np.nonzero(inB & ~inA)[0])
                for i in np.nonzero(flex)[0]:
                    if NA - len(a_list) >= NB - len(b_list):
                        a_list.append(i)
                    else:
                        b_list.append(i)
                if len(a_list) > NA or len(b_list) > NB:
                    raise RuntimeError(
                        f"tile overflow l{lay} core{k} tile{t}: "
                        f"{len(a_list)}/{NA} {len(b_list)}/{NB}")
                # pad slots: negative tail indices -> the gather skips them
                # (their SBUF slots stay stale; dstloc=-1 masks their
                # contribution and a one-time memset guards against NaNs)
                idxA[lay, k, t, len(a_list):] = -1
                idxB[lay, k, t, len(b_list):] = -1
                for p, i in enumerate(a_list):
                    e_id = sel[i]
                    idxA[lay, k, t, p] = ss[i]
                    idxD[lay, k, t, p] = t * 128 + dlane[e_id]
                    dloc[lay, k, t, p] = dlane[e_id]
                for p, i in enumerate(b_list):
                    e_id = sel[i]
                    idxB[lay, k, t, p] = ss[i] - cfg.vb
                    idxD[lay, k, t, NA + p] = t * 128 + dlane[e_id]
                    dloc[lay, k, t, NA + p] = dlane[e_id]

    def pack16(v, width):
        # v [.., NT, width] int -> [.., NT, 128, width//16] int16 wrapped+replicated
        assert v.shape[-1] == width and width % 16 == 0
        lead = v.shape[:-2]
        r = v.reshape(*lead, -1, width // 16, 16)
        r = np.moveaxis(r, -1, -2)      # [.., NT, 16, width//16]
        reps = (1,) * (len(lead) + 1) + (8, 1)
        r = np.tile(r, reps).astype(np.int16)
        return np.ascontiguousarray(r)

    pidxA = pack16(idxA, NA)
    pidxB = pack16(idxB, NB)
    pidxD = pack16(idxD, NA + NB)

    import ml_dtypes
    # dstloc [2, NC, NT, 128, NCH, 2]: position p = j*128 + lane; pads -> -1.
    # Each value is duplicated into a stride-1 pair so the one-hot is_equal
    # can present a stride-1 last dim on every operand (DVE 2x mode).
    dloc_t = np.moveaxis(dloc.reshape(2, NC, NT, NCH, 128), -1, -2)
    dloc_t = np.repeat(dloc_t[..., None], 2, axis=-1)
    dloc_t = np.ascontiguousarray(dloc_t.astype(ml_dtypes.bfloat16))

    # validmask [NC, NT, 128, 1] indexed by (core, tile, lane)
    vm = np.zeros((NC, NT, 128, 1), dtype=np.float32)
    for k in range(NC):
        for t in range(NT):
            vm[k, t, :, 0] = slot2node[k * SH + t * 128 + np.arange(128)] >= 0
    vm = np.ascontiguousarray(vm)

    # x permuted to global row order, then per-core rotation of rank blocks
    xp = np.zeros((cfg.nslot, cfg.fin), dtype=np.float32)
    real = slot2node >= 0
    xp[real] = np.asarray(x, dtype=np.float32)[slot2node[real]]
    xp = xp.astype(ml_dtypes.bfloat16)
    xTs = []
    for k in range(NC):
        xk = np.roll(xp.reshape(NC, SH, cfg.fin), -k, axis=0).reshape(
            cfg.nslot, cfg.fin)
        xTs.append(np.ascontiguousarray(xk.T))

    def chmaj(M, hdim, axis):
        # permute an (h, c)-ordered head block to (c, h) order along axis
        M = np.moveaxis(M, axis, 0)
        M = M.reshape(cfg.h, hdim, *M.shape[1:])
        M = np.swapaxes(M, 0, 1).reshape(cfg.h * hdim, *M.shape[2:])
        return np.moveaxis(M, 0, axis)

    def fuse(W, asv, adv, hdim):
        Wa = np.einsum("fhc,hc->fh", W.reshape(W.shape[0], cfg.h, hdim), asv)
        Wd = np.einsum("fhc,hc->fh", W.reshape(W.shape[0], cfg.h, hdim), adv)
        # attn columns first so the dense phase can copy [attn|h] in one op;
        # h-part columns in (c h) order so the DVE msg multiply gets
        # stride-1 last dims on every operand (2x/4x fast path)
        return np.concatenate([Wa, Wd, chmaj(W, hdim, 1)], axis=1)

    Wall1 = fuse(np.asarray(W1, np.float32), np.asarray(as1, np.float32),
                 np.asarray(ad1, np.float32), cfg.c).astype(ml_dtypes.bfloat16)
    Wall2f = fuse(chmaj(np.asarray(W2, np.float32), cfg.c, 0),
                  np.asarray(as2, np.float32),
                  np.asarray(ad2, np.float32), cfg.out)
    Wall2 = np.ascontiguousarray(
        Wall2f.reshape(2, 128, Wall2f.shape[1])).astype(ml_dtypes.bfloat16)

    b1t = np.ascontiguousarray(np.tile(
        chmaj(np.asarray(b1, np.float32), cfg.c, 0),
        (128, 1))).astype(ml_dtypes.bfloat16)
    b2t = np.ascontiguousarray(np.tile(np.asarray(b2, np.float32), (128, 1)))
    iota = np.tile(np.arange(128, dtype=np.float32), (128, 1)).astype(ml_dtypes.bfloat16)
    ident = np.eye(128, dtype=np.float32).astype(ml_dtypes.bfloat16)

    # out2 row (k, t*128+p) -> node id (-1 for padding lanes)
    out_node = np.full((NC, SH), -1, dtype=np.int64)
    for k in range(NC):
        out_node[k] = slot2node[k * SH:(k + 1) * SH]

    in_maps = []
    for k in range(NC):
        in_maps.append({
            "xT": xTs[k], "Wall1": Wall1, "Wall2": Wall2,
            "b1t": b1t, "b2t": b2t, "iota": iota, "ident": ident,
            "idxA1": pidxA[0, k], "idxB1": pidxB[0, k], "idxD1": pidxD[0, k],
            "idxA2": pidxA[1, k], "idxB2": pidxB[1, k], "idxD2": pidxD[1, k],
            "dstloc1": dloc_t[0, k], "dstloc2": dloc_t[1, k],
            "vmask": vm[k],
        })
    return in_maps, out_node


def build(cfg: Cfg):
    import concourse.bacc as bacc
    import concourse.mybir as mybir
    import concourse.tile as tile
    from concourse import library_config
    from contextlib import ExitStack

    f32 = mybir.dt.float32
    bf16 = mybir.dt.bfloat16
    i16 = mybir.dt.int16
    AOP = mybir.AluOpType
    ACTF = mybir.ActivationFunctionType
    X = mybir.AxisListType.X

    NC, NT, NCH, NA, NB = cfg.ncores, cfg.ntile, cfg.nchunk, cfg.na, cfg.nb
    NAC, NBC = NA // 128, NB // 128
    SH, NSLOT, VA, VB = cfg.shslots, cfg.nslot, cfg.va, cfg.vb
    D1, REC, BT = cfg.d1, cfg.rec, cfg.bt
    HE = 16 + D1          # written row prefix (bf16 cols) = 272
    NW = D1 + 16          # fused weight cols = 272
    RB = 2 * REC          # row length in bf16 units (288 = 576B)
    EPS = 1e-16
    CCB = cfg.ccb

    nc = bacc.Bacc('TRN2', target_bir_lowering=False, debug=False, num_devices=NC)

    xT_d = nc.dram_tensor('xT', [128, NSLOT], bf16, kind='ExternalInput')
    Wall1_d = nc.dram_tensor('Wall1', [128, NW], bf16, kind='ExternalInput')
    Wall2_d = nc.dram_tensor('Wall2', [2, 128, NW], bf16, kind='ExternalInput')
    b1t_d = nc.dram_tensor('b1t', [128, D1], bf16, kind='ExternalInput')
    b2t_d = nc.dram_tensor('b2t', [128, cfg.out], f32, kind='ExternalInput')
    iota_d = nc.dram_tensor('iota', [128, 128], bf16, kind='ExternalInput')
    ident_d = nc.dram_tensor('ident', [128, 128], bf16, kind='ExternalInput')
    idx_d = {}
    for lay in (1, 2):
        idx_d[f'idxA{lay}'] = nc.dram_tensor(
            f'idxA{lay}', [NT, 128, NA // 16], i16, kind='ExternalInput')
        idx_d[f'idxB{lay}'] = nc.dram_tensor(
            f'idxB{lay}', [NT, 128, NB // 16], i16, kind='ExternalInput')
        idx_d[f'idxD{lay}'] = nc.dram_tensor(
            f'idxD{lay}', [NT, 128, (NA + NB) // 16], i16, kind='ExternalInput')
        idx_d[f'dstloc{lay}'] = nc.dram_tensor(
            f'dstloc{lay}', [NT, 128, NCH, 2], bf16, kind='ExternalInput')
    vmask_d = nc.dram_tensor('vmask', [NT, 128, 1], f32, kind='ExternalInput')
    out2_d = nc.dram_tensor('out2', [SH, cfg.out], f32, kind='ExternalOutput')
    T1 = nc.dram_tensor('T1', [NSLOT, RB], bf16, kind='Internal')
    # own-shard [asrc|adst|h-head] cols, written early in the dense phase so
    # the layer-1 dst-attention prefetch overlaps the rest of dense
    ownT = nc.dram_tensor('ownT', [SH, 128], bf16, kind='Internal')
    cc_all = nc.dram_tensor('cc_all', [SH, RB], bf16, kind='Internal')
    T2 = nc.dram_tensor('T2', [NSLOT, RB], bf16, kind='Internal',
                        addr_space='Shared' if cfg.use_collective else 'Local')

    with tile.TileContext(nc) as tc, ExitStack() as ctx, \
            nc.allow_low_precision(reason="bf16 epilogue; rel-err gate 2e-2"):
        const = ctx.enter_context(tc.tile_pool(name='const', bufs=1))
        nc.gpsimd.load_library(library_config.mlp)

        w1 = const.tile([128, NW], bf16)
        nc.sync.dma_start(w1[:], Wall1_d[:])
        w2 = const.tile([128, 2, NW], bf16)
        nc.sync.dma_start(w2[:], Wall2_d[:].rearrange("k p w -> p k w"))
        b1 = const.tile([128, D1], bf16)
        nc.sync.dma_start(b1[:], b1t_d[:])
        b2 = const.tile([128, cfg.out], f32)
        nc.sync.dma_start(b2[:], b2t_d[:])
        iot = const.tile([128, 128], bf16)
        nc.sync.dma_start(iot[:], iota_d[:])
        idn = const.tile([128, 128], bf16)
        nc.sync.dma_start(idn[:], ident_d[:])
        vmt = const.tile([128, NT], f32)
        nc.sync.dma_start(vmt[:], vmask_d[:].rearrange("t p o -> p (t o)"))

        # persistent pool for the dst-attention prefetch gathers: allocated
        # BEFORE the dense-phase pools so its SBUF does not alias theirs
        # (aliasing would add an anti-dependency that kills the overlap)
        pfp = ctx.enter_context(tc.tile_pool(name='pf', bufs=2))
        PBT = 4

        # ---------- edge phases ----------
        idxp = ctx.enter_context(tc.tile_pool(name='idx', bufs=1))

        def load_idx(lay):
            iaL = idxp.tile([128, NT, NA // 16], i16, name=f'iaL{lay}')
            nc.sync.dma_start(iaL[:], idx_d[f'idxA{lay}'][:].rearrange("t p w -> p t w"))
            ibL = idxp.tile([128, NT, NB // 16], i16, name=f'ibL{lay}')
            nc.sync.dma_start(ibL[:], idx_d[f'idxB{lay}'][:].rearrange("t p w -> p t w"))
            idL = idxp.tile([128, NT, (NA + NB) // 16], i16, name=f'idL{lay}')
            nc.sync.dma_start(idL[:], idx_d[f'idxD{lay}'][:].rearrange("t p w -> p t w"))
            dlL = idxp.tile([128, NT, NCH, 2], bf16, name=f'dlL{lay}')
            nc.sync.dma_start(dlL[:], idx_d[f'dstloc{lay}'][:].rearrange(
                "t p w two -> p t w two"))
            return iaL, ibL, idL, dlL

        idx1 = load_idx(1)

        # per-edge dst attention, compacted to 8 cols: zda[lay][:, t, j, h]
        zda = {1: idxp.tile([128, NT, NCH, 8], bf16, name='zda1'),
               2: idxp.tile([128, NT, NCH, 8], bf16, name='zda2')}


        # ---------- phase D1: replicated dense, writes T1 (local order) ----
        with tc.tile_pool(name='dx', bufs=2) as dx, \
             tc.tile_pool(name='dps', bufs=4, space='PSUM') as dps, \
             tc.tile_pool(name='dstg', bufs=3) as dstg:
            ng = NSLOT // 128
            for g0 in range(0, ng, cfg.xbatch):
                gb = min(cfg.xbatch, ng - g0)
                xt = dx.tile([128, gb * 128], bf16, tag='xt')
                nc.sync.dma_start(xt[:], xT_d[:, g0 * 128:(g0 + gb) * 128])
                for w0 in range(0, gb, cfg.wg):
                    wg = min(cfg.wg, gb - w0)
                    stg = dstg.tile([128, wg, HE], bf16, tag=f'stg{wg}')
                    for t in range(wg):
                        ps = dps.tile([128, NW], f32, tag='dps')
                        nc.tensor.matmul(ps[:], xt[:, (w0 + t) * 128:(w0 + t + 1) * 128],
                                         w1[:], start=True, stop=True)
                        if t % 2 == 0:
                            nc.scalar.copy(stg[:, t, :], ps[:])
                        else:
                            nc.vector.tensor_copy(stg[:, t, :], ps[:])
                    g = g0 + w0
                    nc.sync.dma_start(
                        T1[g * 128:(g + wg) * 128, 0:HE].rearrange(
                            "(t p) r -> p t r", p=128),
                        stg[:])
                    if g < NT:
                        gc = min(wg, NT - g)
                        nc.sync.dma_start(
                            ownT[g * 128:(g + gc) * 128, :].rearrange(
                                "(t p) r -> p t r", p=128),
                            stg[:, 0:gc, 0:128])

        def prefetch_gd_batch(layer, adst_src, estep, idL, b0):
            """Gather adst[dst] for one batch of edge slots (256B elems from
            the local own-shard rows; cols 8:16 are the payload) and compact
            into zda. Runs on Pool/DMA only."""
            bt = min(PBT, NT - b0)
            gD = pfp.tile([128, PBT * NCH, 128], bf16, tag='gD')
            nc.gpsimd.dma_gather(
                gD[:, 0:bt * NCH, :], adst_src,
                idL[:, b0:b0 + bt, :].rearrange("p t w -> p (t w)"),
                bt * (NA + NB), bt * (NA + NB), 128, elem_step=estep,
                single_packet=False)
            nc.scalar.copy(
                zda[layer][:, b0:b0 + bt, :, :],
                gD[:, 0:bt * NCH, 8:16].rearrange(
                    "p (t j) h -> p t j h", t=bt))

        def edge_phase(layer, T, idxt, epilogue, prefetch_cb=None):
            iaL, ibL, idL, dlL = idxt
            zd = zda[layer]
            pname = f'e{layer}'
            pend = [None]
            GBUFS = 4
            with tc.tile_pool(name=pname + 'g', bufs=GBUFS) as gp, \
                 tc.tile_pool(name=pname + 'w', bufs=4) as wp, \
                 tc.tile_pool(name=pname + 'o', bufs=4) as op, \
                 tc.tile_pool(name=pname + 'ps', bufs=4, space='PSUM') as pp:
                rowA_src = T[0:VA, :]
                rowB_src = T[VB:NSLOT, :]

                # one-time memset of every gather buffer: pad tail slots are
                # skipped by the DMA (negative idx) and must not hold NaNs
                # (0 x NaN would poison the PSUM scatter)
                for _ in range(GBUFS):
                    gz = gp.tile([128, NCH, RB], bf16, tag='gA')
                    nc.gpsimd.memset(gz[:], 0.0)

                if True:
                    for tg in range(NT):
                        if prefetch_cb is not None and tg % PBT == 0:
                            prefetch_cb(tg)
                        # both view gathers land in ONE tile (disjoint row
                        # ranges) so zb/msg each run as a single DVE op
                        g = gp.tile([128, NCH, RB], bf16, tag='gA')
                        nc.gpsimd.dma_gather(
                            g[:, 0:NAC, :], rowA_src,
                            iaL[:, tg, :],
                            NA, NA, RB, single_packet=False)
                        nc.gpsimd.dma_gather(
                            g[:, NAC:NCH, :], rowB_src,
                            ibL[:, tg, :],
                            NB, NB, RB, single_packet=False)
                        # one-hots for all chunks of this tile: oh[e, j, slot]
                        ohs = wp.tile([128, NCH, 128], bf16, tag='ohs')
                        # stride-1 pair view on every operand -> DVE 2x mode
                        nc.vector.tensor_tensor(
                            ohs[:].rearrange("p j (s two) -> p j s two", two=2),
                            iot[:].rearrange("p (s two) -> p () s two", two=2)
                            .to_broadcast([128, NCH, 64, 2]),
                            dlL[:, tg, :, :].rearrange("p j two -> p j () two")
                            .to_broadcast([128, NCH, 64, 2]),
                            op=AOP.is_equal)
                        # z = asrc[src] + adst[dst]; leaky; exp
                        zb = wp.tile([128, NCH * 8], bf16, tag='zb')
                        nc.vector.tensor_tensor(
                            zb[:].rearrange("p (b h) -> p b h", b=NCH),
                            g[:, :, 0:8],
                            zd[:, tg, :, :],
                            op=AOP.add)
                        zl = wp.tile([128, NCH * 8], bf16, tag='zl')
                        nc.vector.scalar_tensor_tensor(
                            zl[:], zb[:], 0.2, zb[:], op0=AOP.mult, op1=AOP.max)
                        p = wp.tile([128, NCH * 8], bf16, tag='p')
                        nc.scalar.activation(p[:], zl[:], ACTF.Exp)
                        # msg = h[src] * p  (h stored (c h)-major: all
                        # operands stride-1 in the last dim -> DVE fast path)
                        msg = wp.tile([128, NCH, 32, 8], bf16, tag='msgA')
                        nc.vector.tensor_tensor(
                            msg[:],
                            g[:, :, 16:16 + D1].rearrange(
                                "p b (c h) -> p b c h", h=8),
                            p[:].rearrange(
                                "p (b h) -> p b () h", b=NCH).to_broadcast(
                                [128, NCH, 32, 8]),
                            op=AOP.mult)
                        # scatter to dst slots
                        paw = pp.tile([128, D1], f32, tag='paw')
                        pdt = pp.tile([128, 8], f32, tag='aux', name='pdt')
                        for j in range(NCH):
                            rhs = msg[:, j].rearrange("p c h -> p (c h)")
                            nc.tensor.matmul(paw[:], ohs[:, j, :], rhs,
                                             start=(j == 0), stop=(j == NCH - 1))
                            nc.tensor.matmul(
                                pdt[:], ohs[:, j, :], p[:, j * 8:(j + 1) * 8],
                                start=(j == 0), stop=(j == NCH - 1))
                        # software-pipelined epilogue: defer by one tile so
                        # the in-order DVE never head-of-line blocks on this
                        # tile's PSUM while the next tile's front work is ready
                        if pend[0] is not None:
                            epilogue(*pend[0])
                        pend[0] = (tg, paw[:], pdt[:], op, pp)
                if pend[0] is not None:
                    epilogue(*pend[0])
                    pend[0] = None

        # ---------- epilogues ----------
        ccstage = {}

        def epi1(tg, pa, pd, op, pp):
            d1 = op.tile([128, 8], f32, tag='d1')
            # PSUM-near Act engine does the EPS add (Copy table is
            # always resident); keeps the DVE chain off PSUM reads
            nc.scalar.activation(d1[:], pd, ACTF.Copy, bias=EPS)
            r = op.tile([128, 8], bf16, tag='r')
            nc.vector.reciprocal(r[:], d1[:])
            # PSUM read on Act (bf16 out) so the DVE chain stays in 2x mode
            pab = op.tile([128, D1], bf16, tag='pab')
            nc.scalar.copy(pab[:], pa)
            o1 = op.tile([128, D1], bf16, tag='o1')
            rb = r[:].rearrange("p h -> p () h").to_broadcast([128, 32, 8])
            nc.vector.tensor_tensor(o1[:].rearrange("p (c h) -> p c h", h=8),
                                    pab[:].rearrange("p (c h) -> p c h", h=8),
                                    rb, op=AOP.mult)
            nc.vector.tensor_tensor(o1[:], o1[:], b1[:], op=AOP.add)
            ex = op.tile([128, D1], bf16, tag='ex')
            nc.scalar.activation(ex[:], o1[:], ACTF.Exp)
            nc.vector.tensor_scalar(ex[:], ex[:], 1.0, 1.0, op0=AOP.min,
                                    op1=AOP.subtract)
            et = op.tile([128, D1], bf16, tag='et')
            nc.vector.scalar_tensor_tensor(
                et[:], o1[:], 0.0, ex[:], op0=AOP.max, op1=AOP.add)
            # ---- fused D2: h2 row for this tile -> cc staging ----
            lh = op.tile([128, 2, 128], bf16, tag='lh')
            ptr = pp.tile([128, 2, 128], bf16, tag='aux', name='ptr')
            nc.tensor.transpose(ptr[:, 0], et[:, 0:128], idn[:])
            nc.tensor.transpose(ptr[:, 1], et[:, 128:256], idn[:])
            nc.scalar.copy(lh[:], ptr[:])
            pd2 = pp.tile([128, NW], f32, tag='aux', name='pd2')
            nc.tensor.matmul(pd2[:], lh[:, 0], w2[:, 0], start=True, stop=False)
            nc.tensor.matmul(pd2[:], lh[:, 1], w2[:, 1], start=False, stop=True)
            # per-tile cc write (544B payload cols only; cc_all pad cols stay
            # DRAM-stale, nothing consumes them): the last tile's rows reach
            # DRAM right after its epilogue, so the AllGather starts sooner
            row = op.tile([128, HE], bf16, tag='ccstg')
            nc.scalar.activation(row[:], pd2[:],
                                 ACTF.Copy, scale=vmt[:, tg:tg + 1])
            nc.sync.dma_start(
                cc_all[tg * 128:(tg + 1) * 128, 0:HE].rearrange(
                    "(t p) r -> p (t r)", p=128), row[:])

        outstage = {}

        def epi2(tg, pa, pd, op, pp):
            d1 = op.tile([128, 8], f32, tag='d1')
            # PSUM-near Act engine does the EPS add (Copy table is
            # always resident); keeps the DVE chain off PSUM reads
            nc.scalar.activation(d1[:], pd, ACTF.Copy, bias=EPS)
            r = op.tile([128, 8], bf16, tag='r')
            nc.vector.reciprocal(r[:], d1[:])
            pab = op.tile([128, D1], bf16, tag='pab')
            nc.scalar.copy(pab[:], pa)
            o1 = op.tile([128, D1], bf16, tag='o1')
            rb = r[:].rearrange("p h -> p () h").to_broadcast([128, cfg.out, 8])
            nc.vector.tensor_tensor(o1[:].rearrange("p (c h) -> p c h", h=8),
                                    pab[:].rearrange("p (c h) -> p c h", h=8),
                                    rb, op=AOP.mult)
            m = op.tile([128, cfg.out], f32, tag='m')
            nc.vector.reduce_sum(m[:].rearrange("p c -> p c ()"),
                                 o1[:].rearrange("p (c h) -> p c h", h=8), axis=X)
            # per-tile out write: the kernel end isn't gated on a 7-tile
            # staging group after the last epilogue
            ob = op.tile([128, cfg.out], f32, tag='ostg')
            nc.vector.scalar_tensor_tensor(ob[:], m[:], 1.0 / cfg.h,
                                           b2[:], op0=AOP.mult, op1=AOP.add)
            nc.sync.dma_start(
                out2_d[tg * 128:(tg + 1) * 128, :].rearrange(
                    "(t p) c -> p (t c)", p=128), ob[:])

        # layer-1 dst-attention prefetch: the first batches depend only on
        # the early ownT writes and overlap the rest of the dense phase;
        # later batches issue inside the L1 loop (two batches ahead) so
        # their DMA rides L1's slack instead of delaying the T1 writes
        # that gate L1's first row gathers
        prefetch_gd_batch(1, ownT[0:SH, :], 128, idx1[2], 0)
        prefetch_gd_batch(1, ownT[0:SH, :], 128, idx1[2], PBT)

        def pf1_cb(b0):
            nxt = b0 + 2 * PBT
            if nxt < NT:
                prefetch_gd_batch(1, ownT[0:SH, :], 128, idx1[2], nxt)

        edge_phase(1, T1, idx1, epi1, prefetch_cb=pf1_cb)

        if cfg.use_collective:
            nc.gpsimd.collective_compute(
                "AllGather", mybir.AluOpType.bypass,
                ins=[cc_all[:]],
                outs=[T2[:]],
                replica_groups=[list(range(NC))],
            )
        else:
            # timing-sim-only stand-in for the AllGather: copy local rows to
            # every rank block of T2 (values wrong cross-core, local DMA cost
            # similar to the receive side of the real collective)
            with tc.tile_pool(name='ccb', bufs=2) as ccbp:
                for cci in range(NT // CCB):
                    bb = ccbp.tile([128, CCB, RB], bf16, tag='bb')
                    nc.sync.dma_start(
                        bb[:], cc_all[cci * CCB * 128:(cci + 1) * CCB * 128,
                                      :].rearrange("(t p) r -> p t r", p=128))
                    for k in range(NC):
                        nc.sync.dma_start(
                            T2[k * SH + cci * CCB * 128:
                               k * SH + (cci + 1) * CCB * 128, :].rearrange(
                                "(t p) r -> p t r", p=128), bb[:])

        # layer-2 index loads + dst-attention prefetch: issued after the
        # AllGather so both run inside its window (the prefetch reads only
        # the local cc staging)
        idx2 = load_idx(2)
        for b0 in range(0, NT, PBT):
            prefetch_gd_batch(2, cc_all[0:SH, 0:128], RB, idx2[2], b0)

        edge_phase(2, T2, idx2, epi2)

    nc.compile()
    return nc


_CACHE = {}


def kernel(x, edge_index, W1, att_src1, att_dst1, b1, W2, att_src2,
           att_dst2, b2):
    cfg = Cfg()
    in_maps, out_node = host_prep(cfg, x, edge_index, W1, att_src1,
                                  att_dst1, b1, W2, att_src2, att_dst2, b2)
    if 'nc' not in _CACHE:
        _CACHE['nc'] = build(cfg)
    nc = _CACHE['nc']
    from concourse.bass_utils import run_bass_kernel_spmd
    res = run_bass_kernel_spmd(nc, in_maps, core_ids=list(range(cfg.ncores)))
    full = np.concatenate([res.results[k]['out2'] for k in range(cfg.ncores)],
                          axis=0)
    flat = out_node.reshape(-1)
    out = np.zeros((cfg.n, cfg.out), np.float32)
    real = flat >= 0
    out[flat[real]] = full[real]
    return out
